# revision 41
# baseline (speedup 1.0000x reference)
"""Trainium2 Bass kernel for MultiHeadSelfAttention.

Full inputs -> shard across 8 NeuronCores (batch x head-group) -> SPMD Bass
kernel -> gather partial projections on host.

Per-core shard (core c): batch b = c//4, head group g = c%4 (4 heads of 16).

Device program (per core, T=2048, D=1024, 4 heads, dh=64):
  phase A (per 512-col chunk s): kT,qT = (Wqk.T @ x chunk) [feat,T] f32r,
    kT,qT stored bf16; v = x @ Wv (natural [T,feat]) bf16 + ones col;
    chunk-0 attention units with tk-chunk == s run immediately (pair0
    fully; pair1 exp/mask staged, V deferred) so ACT/DVE stay busy through
    the QKV phase.
  attention unit (head h, group g = 2 tk-tiles): S^T = kT.T @ qT (bf16,
    K=64; the two heads of a pair sit in opposite partition halves, giving
    the PE row-tiling headroom).  E is produced either
      exact: ACT exp (scale ln2/128, since qT carries A2=16*log2e) -> bf16,
             then keep-mask multiply (bf16 2x) on DVE or GpSimd, or
      trick:  one DVE scalar_tensor_tensor (S' + B2) * keep -> int16 whose
             bits ARE bf16 exp(S/8)*keep (Schraudolph in bf16-bit space).
  O^T accumulated per head in PSUM [65,512] (ones-row = softmax denom d);
  normalize: 1/d via DVE fast reciprocal straight from PSUM, K=1 matmul
  broadcast, mul (writes ot_stk rows directly for even heads, DMA restack
  for odd); projection out = O_cat^T.T @ Wp emitted one chunk late.
All matmuls bf16 or f32r; a junk-matmul warmup at t=0 pre-warms the PE HAM
clock gate while the first weight DMAs stream in.
"""

import os
import sys

import numpy as np

sys.path.insert(0, "/opt/trn_rl_repo")

import ml_dtypes  # noqa: E402

import concourse.mybir as mybir  # noqa: E402
import concourse.tile as tile  # noqa: E402
from concourse import bacc  # noqa: E402
from concourse.bass import ts  # noqa: E402

D = 1024  # d_model
HC = 4  # heads per core
DH = 64  # head dim
FQ = HC * DH  # 256: per-core q (or k or v) feature count
T_FULL = 2048
CHUNK = 512
NK = D // 128  # contraction tiles over d_model
NCH = T_FULL // CHUNK  # 4 chunks
NTT = T_FULL // 128  # 16 tk tiles
TPC = CHUNK // 128  # 4 tiles per chunk
NG = NTT // 2  # 8 groups (units) of 2 tk-tiles per (head, chunk)

F32 = mybir.dt.float32
F32R = mybir.dt.float32r
BF16 = mybir.dt.bfloat16
I16 = mybir.dt.int16
EXP = mybir.ActivationFunctionType.Exp
ADD = mybir.AluOpType.add
MULT = mybir.AluOpType.mult

A2 = 16.0 / np.log(2.0)  # folded into Wq on host: S' = S_raw * A2
B2 = 16256.0 - 5.5  # bf16-bit exp shift (round-to-nearest converts)
EXP_SCALE = float(1.0 / (8.0 * A2))  # ln2/128: exp(S'/(8*A2)) = exp(S/8)

# unit routing per (pair, head): g in 0..7
TRICK_G = frozenset((1, 6))  # fused DVE bit-exp (25%)
GPS_G = frozenset((0, 3, 5))  # exact; keep-mask multiply on GpSimd
# remaining exact groups {2, 7} mask on DVE
N_WARM = 40  # junk warmup matmuls (HAM pre-warm)


def build_program(T=T_FULL):
    nc = bacc.Bacc("TRN2", target_bir_lowering=False, debug=False)

    xt_d = nc.dram_tensor("xt", [D, T], BF16, kind="ExternalInput")
    wqk_d = nc.dram_tensor("wqk", [D, 2 * FQ], BF16, kind="ExternalInput")
    wv_d = nc.dram_tensor("wv", [D, FQ], BF16, kind="ExternalInput")
    wp_d = nc.dram_tensor("wp", [FQ, D], F32R, kind="ExternalInput")
    bqk_d = nc.dram_tensor("bqk", [128, 4], F32, kind="ExternalInput")
    bv_d = nc.dram_tensor("bv", [128, FQ], BF16, kind="ExternalInput")
    kp_d = nc.dram_tensor("keept", [T, T], BF16, kind="ExternalInput")
    out_d = nc.dram_tensor("out", [T, D], F32, kind="ExternalOutput")
    DBG = os.environ.get("KDBG", "0") == "1"
    if DBG:
        dbg_kt = nc.dram_tensor("dbg_kt", [128, 2, CHUNK], F32, kind="ExternalOutput")
        dbg_qt = nc.dram_tensor("dbg_qt", [128, 2, CHUNK], F32, kind="ExternalOutput")
        dbg_v = nc.dram_tensor("dbg_v", [128, TPC, HC, DH + 2], BF16, kind="ExternalOutput")
        dbg_e = nc.dram_tensor("dbg_e", [128, 2, CHUNK], BF16, kind="ExternalOutput")
        dbg_rb = nc.dram_tensor("dbg_rb", [64, CHUNK], F32, kind="ExternalOutput")
        dbg_d = nc.dram_tensor("dbg_d", [DH + 1, CHUNK], F32, kind="ExternalOutput")
        dbg_ot = nc.dram_tensor("dbg_ot", [128, 2, T], F32, kind="ExternalOutput")

    xt_r = xt_d.rearrange("(a p) t -> p a t", p=128)  # [128, NK, T]
    wqk_r = wqk_d.rearrange("(a p) f -> p a f", p=128)  # [128, NK, 512]
    wv_r = wv_d.rearrange("(a p) f -> p a f", p=128)  # [128, NK, 256]
    wp_r = wp_d.rearrange("(a p) n -> p a n", p=128)  # [128, 2, D]
    kp_r = kp_d.rearrange("(a p) q -> p a q", p=128)  # [128, NTT, T]

    with tile.TileContext(nc) as tc:
        with (
            tc.tile_pool(name="const", bufs=1) as constp,
            tc.tile_pool(name="stream", bufs=2) as streamp,
            tc.tile_pool(name="qkv", bufs=1) as qkvp,
            tc.tile_pool(name="work", bufs=2) as workp,
            tc.tile_pool(name="ps", bufs=2, space="PSUM") as psp,
        ):
            # ---- constants ----
            ones_f = constp.tile([1, 128], F32, name="ones_f")
            nc.vector.memset(ones_f[:], 1.0)
            ones64 = constp.tile([128, 64], BF16, name="ones64")
            nc.vector.memset(ones64[:], 1.0)
            bqk_sb = constp.tile([128, 4], F32, name="bqk_sb")
            nc.sync.dma_start(bqk_sb[:], bqk_d[:, :])
            bv_bc = constp.tile([128, FQ], BF16, name="bv_bc")
            nc.sync.dma_start(bv_bc[:], bv_d[:, :])

            # ---- HAM warmup: junk matmuls on memset data ----
            junk_wf = constp.tile([128, 128], F32, name="junk_wf")
            nc.vector.memset(junk_wf[:], 0.5)
            junk_w = constp.tile([128, 128], F32R, name="junk_w")
            nc.vector.tensor_copy(junk_w[:], junk_wf[:])
            junk_rf = constp.tile([128, 512], F32, name="junk_rf")
            nc.vector.memset(junk_rf[:], 0.25)
            junk_r = constp.tile([128, 512], F32R, name="junk_r")
            nc.vector.tensor_copy(junk_r[:], junk_rf[:])
            for _ in range(N_WARM):
                ps_j = psp.tile([128, 512], F32, tag="F", name="ps_j")
                nc.tensor.matmul(
                    ps_j[:], junk_w[:], junk_r[:], start=True, stop=True
                )

            # ---- weights (sliced so QKV(0) can start early) ----
            wqk_sb = constp.tile([128, NK, 2 * FQ], BF16, name="wqk_sb")
            xt_tiles = []
            xt0 = streamp.tile([128, NK, CHUNK], BF16, tag="xt", bufs=3,
                               name="xt0")
            for k in range(NK):
                nc.sync.dma_start(wqk_sb[:, k, :], wqk_r[:, k, :])
                nc.scalar.dma_start(xt0[:, k, :], xt_r[:, k, ts(0, CHUNK)])
            xt_pre = {0: xt0}
            xt1 = streamp.tile([128, NK, CHUNK], BF16, tag="xt", bufs=3,
                               name="xt1")
            nc.gpsimd.dma_start(xt1[:, :, :], xt_r[:, :, ts(1, CHUNK)])
            kp0 = streamp.tile([128, NTT, CHUNK], BF16, tag="kp", bufs=2,
                               name="kp0")
            nc.gpsimd.dma_start(kp0[:, 0 : NTT // 2, :],
                                kp_r[:, 0 : NTT // 2, ts(0, CHUNK)])
            nc.gpsimd.dma_start(kp0[:, NTT // 2 :, :],
                                kp_r[:, NTT // 2 :, ts(0, CHUNK)])
            xt_pre[1] = xt1
            wv_sb = constp.tile([128, NK, FQ], BF16, name="wv_sb")
            nc.sync.dma_start(wv_sb[:], wv_r[:, :, :])
            wp_sb = constp.tile([128, FQ // 128, D], F32R, name="wp_sb")

            ot_stk = constp.tile([128, FQ // 128, T], F32R, name="ot_stk")

            kt_tiles, qt_tiles, v_tiles = [], [], []
            kp_tiles = {0: kp0}
            po = {}  # (pair, head) -> psum tile, per live pair
            pend = []  # attention units awaiting V emission
            staged = []  # phase-A pair1 units with V deferred

            def emit_s(c, p, h_pair, jj, etag="e", ebufs=12):
                """One tk-tile unit covering BOTH heads of the pair: the two
                K=64 S matmuls write one shared PSUM tile (row-tiled halves)
                and a single exp/trick op releases it, so the next pair of S
                matmuls becomes ready together and co-issues on the PE."""
                cc, tt = divmod(jj, TPC)
                ps = psp.tile([128, 2, CHUNK], F32, tag="S", name="ps_s")
                for idx, h in enumerate(h_pair):
                    hb = 64 * (h % 2)
                    hf = h // 2
                    nc.tensor.matmul(
                        ps[:, idx, :],
                        kt_tiles[cc][hb : hb + 64, hf, ts(tt, 128)],
                        qt_tiles[c][hb : hb + 64, hf, :],
                        start=True, stop=True,
                    )
                g = jj // 2
                kp_s = kp_tiles[c][:, jj, :]
                if g in TRICK_G:
                    e_t = workp.tile([128, 2, CHUNK], I16, tag=etag,
                                     bufs=ebufs, name="e_i16")
                    for idx in range(2):
                        nc.vector.scalar_tensor_tensor(
                            e_t[:, idx, :], ps[:, idx, :], B2, kp_s,
                            op0=ADD, op1=MULT)
                    rhs = e_t[:, :, :].bitcast(BF16)
                else:
                    e_t = workp.tile([128, 2, CHUNK], BF16, tag=etag,
                                     bufs=ebufs, name="e_bf")
                    nc.scalar.activation(e_t[:], ps[:], EXP, scale=EXP_SCALE)
                    meng = nc.gpsimd if g in GPS_G else nc.vector
                    for idx in range(2):
                        meng.tensor_mul(e_t[:, idx, :], e_t[:, idx, :], kp_s)
                    rhs = e_t[:, :, :]
                return [(c, p, h_pair, jj, rhs)]

            def emit_v(unit):
                c, p, h_pair, jj, rhs = unit
                cc, tt = divmod(jj, TPC)
                for idx, h in enumerate(h_pair):
                    if jj == 0:
                        po[(p, h)] = psp.tile([128, CHUNK], F32, tag="O",
                                              name=f"po{h}")
                    nc.tensor.matmul(
                        po[(p, h)][0 : DH + 1, :],
                        v_tiles[cc][:, tt, h, 0 : DH + 1],
                        rhs[:, idx, :],
                        start=(jj == 0), stop=(jj == NTT - 1),
                    )
                if jj == NTT - 1:
                    for h in h_pair:
                        emit_norm(c, p, h)

            def emit_norm(c, p, h):
                hb = 64 * (h % 2)
                hf = h // 2
                pt = po.pop((p, h))
                r1 = workp.tile([DH + 1, CHUNK], BF16, tag="r1", bufs=2,
                                name="r1")
                nc.scalar.copy(r1[DH : DH + 1, :], pt[DH : DH + 1, :])
                pb = psp.tile([64, CHUNK], F32, tag="F", name="pb")
                nc.tensor.matmul(pb[:], ones64[DH : DH + 1, :],
                                 r1[DH : DH + 1, :],
                                 start=True, stop=True)
                rb = workp.tile([64, CHUNK], F32, tag="rb", bufs=2, name="rb")
                nc.vector.reciprocal_approx_fast(rb[:], pb[:])
                if DBG and (c, p, h) == (0, 0, 0):
                    nc.sync.dma_start(dbg_rb[:, :], rb[:])
                    nc.sync.dma_start(dbg_d[:, :], r1[:].bitcast(F32))
                if hb == 0:
                    nc.vector.tensor_mul(
                        ot_stk[0:64, hf, ts(c, CHUNK)], pt[0:DH, :], rb[:]
                    )
                else:
                    ot_sb = workp.tile([64, CHUNK], F32R, tag="ot", bufs=2,
                                       name="ot_sb")
                    nc.vector.tensor_mul(ot_sb[:], pt[0:DH, :], rb[:])
                    nc.sync.dma_start(
                        ot_stk[64:128, hf, ts(c, CHUNK)], ot_sb[:]
                    )

            def push_units(units, drain_to):
                pend.extend(units)
                while len(pend) > drain_to:
                    emit_v(pend.pop(0))

            def emit_proj(cp):
                for tt in range(TPC):
                    tglob = cp * TPC + tt
                    o_t = workp.tile([128, D], F32, tag="out", bufs=2,
                                     name="o_t")
                    for n in range(D // CHUNK):
                        ps_f = psp.tile([128, CHUNK], F32, tag="F",
                                        name="ps_f")
                        for j in range(FQ // 128):
                            nc.tensor.matmul(
                                ps_f[:],
                                ot_stk[:, j, ts(tglob, 128)],
                                wp_sb[:, j, ts(n, CHUNK)],
                                start=(j == 0), stop=(j == FQ // 128 - 1),
                            )
                        nc.scalar.copy(o_t[:, ts(n, CHUNK)], ps_f[:])
                    nc.sync.dma_start(out_d[ts(tglob, 128), :], o_t[:])

            # ---- phase A: QKV per chunk + chunk-0 attention staging ----
            for s in range(NCH):
                if s == 1:
                    for _ in range(8):
                        ps_jb = psp.tile([128, 2, CHUNK], F32, tag="S",
                                         name="ps_jb")
                        nc.tensor.matmul(ps_jb[:, 0, :], junk_w[:],
                                         junk_r[:], start=True, stop=True)
                xt_t = xt_pre[s]
                if s + 2 < NCH:
                    xt_n = streamp.tile([128, NK, CHUNK], BF16, tag="xt",
                                        bufs=3, name="xt_n")
                    nc.scalar.dma_start(
                        xt_n[:, 0 : NK // 2, :],
                        xt_r[:, 0 : NK // 2, ts(s + 2, CHUNK)])
                    nc.sync.dma_start(
                        xt_n[:, NK // 2 :, :],
                        xt_r[:, NK // 2 :, ts(s + 2, CHUNK)])
                    xt_pre[s + 2] = xt_n

                kt_t = qkvp.tile([128, 2, CHUNK], BF16, tag="kt", bufs=NCH,
                                 name="kt_t")
                qt_t = qkvp.tile([128, 2, CHUNK], BF16, tag="qt", bufs=NCH,
                                 name="qt_t")
                for f in range(2):  # k features: wqk cols 256..511
                    ps_k = psp.tile([128, CHUNK], F32, tag="F", name="ps_k")
                    for k in range(NK):
                        nc.tensor.matmul(
                            ps_k[:], wqk_sb[:, k, ts(2 + f, 128)],
                            xt_t[:, k, :],
                            start=(k == 0), stop=(k == NK - 1),
                        )
                    nc.vector.tensor_scalar_add(
                        kt_t[:, f, :], ps_k[:], bqk_sb[:, 2 + f : 3 + f]
                    )
                for f in range(2):  # q features (pre-scaled by A2 on host)
                    ps_q = psp.tile([128, CHUNK], F32, tag="F", name="ps_q")
                    for k in range(NK):
                        nc.tensor.matmul(
                            ps_q[:], wqk_sb[:, k, ts(f, 128)],
                            xt_t[:, k, :],
                            start=(k == 0), stop=(k == NK - 1),
                        )
                    nc.vector.tensor_scalar_add(
                        qt_t[:, f, :], ps_q[:], bqk_sb[:, f : f + 1]
                    )

                v_t = qkvp.tile([128, TPC, HC, DH + 2], BF16, tag="v",
                                bufs=NCH, name="v_t")
                nc.vector.memset(
                    v_t[:, :, :, DH : DH + 1].rearrange("p a h e -> p (a h e)"),
                    1.0,
                )
                for tt in range(TPC):
                    ps_v = psp.tile([128, FQ], F32, tag="F", name="ps_v")
                    for k in range(NK):
                        nc.tensor.matmul(
                            ps_v[:], xt_t[:, k, ts(tt, 128)], wv_sb[:, k, :],
                            start=(k == 0), stop=(k == NK - 1),
                        )
                    nc.vector.tensor_add(
                        v_t[:, tt, :, 0:DH],
                        ps_v[:, :].rearrange("p (h e) -> p h e", h=HC),
                        bv_bc[:, :].rearrange("p (h e) -> p h e", h=HC),
                    )
                kt_tiles.append(kt_t)
                qt_tiles.append(qt_t)
                v_tiles.append(v_t)
                if DBG and s == 0:
                    nc.sync.dma_start(dbg_kt[:, :, :], kt_t[:].bitcast(F32))
                    nc.sync.dma_start(dbg_qt[:, :, :], qt_t[:].bitcast(F32))
                    nc.sync.dma_start(dbg_v[:, :, :, :], v_t[:])

                # chunk-0 attention for tk-chunk == s: pair0 live, pair1
                # exp/mask staged (V deferred until phase B).
                for jj in range(4 * s, 4 * s + 4):
                    push_units(emit_s(0, 0, (0, 1), jj), 4)
                    staged.extend(
                        emit_s(0, 1, (2, 3), jj, etag="estage", ebufs=16))
                if s == 1:
                    # projection weights: first use is after chunk-0 norms
                    nc.sync.dma_start(wp_sb[:], wp_r[:, :, :])
                if s == 2:
                    kp1 = streamp.tile([128, NTT, CHUNK], BF16, tag="kp",
                                       bufs=2, name="kp_t")
                    nc.gpsimd.dma_start(kp1[:, 0 : NTT // 2, :],
                                        kp_r[:, 0 : NTT // 2, ts(1, CHUNK)])
                    nc.sync.dma_start(kp1[:, NTT // 2 :, :],
                                      kp_r[:, NTT // 2 :, ts(1, CHUNK)])
                    kp_tiles[1] = kp1

            while pend:  # finish chunk-0 pair0
                emit_v(pend.pop(0))

            # ---- phase B ----
            # chunk 0 pair1: V-sweep over staged units, then proj(0) later.
            for unit in staged:
                emit_v(unit)

            for c in range(1, NCH):
                if c + 1 < NCH:
                    kp_n = streamp.tile([128, NTT, CHUNK], BF16, tag="kp",
                                        bufs=2, name="kp_t")
                    nc.gpsimd.dma_start(
                        kp_n[:, 0 : NTT // 2, :],
                        kp_r[:, 0 : NTT // 2, ts(c + 1, CHUNK)])
                    nc.sync.dma_start(
                        kp_n[:, NTT // 2 :, :],
                        kp_r[:, NTT // 2 :, ts(c + 1, CHUNK)])
                    kp_tiles[c + 1] = kp_n
                for p, h_pair in ((0, (0, 1)), (1, (2, 3))):
                    for jj in range(NTT):
                        dt = 5 if (c == NCH - 1 and p == 1) else 10
                        push_units(emit_s(c, p, h_pair, jj), dt)
                        if p == 0 and jj == 12:
                            emit_proj(c - 1)  # lagged one chunk

            while pend:
                emit_v(pend.pop(0))
            emit_proj(NCH - 1)
            if DBG:
                nc.sync.dma_start(dbg_ot[:, :, :], ot_stk[:].bitcast(F32))

    nc.compile()
    return nc


def shard_inputs(x, mask, Wqkv, bqkv, Wproj):
    """Build the 8 per-core input maps from full inputs."""
    bf16 = ml_dtypes.bfloat16
    x = np.asarray(x, dtype=np.float32)
    Wqkv = np.asarray(Wqkv, dtype=np.float32)
    bqkv = np.asarray(bqkv, dtype=np.float32)
    keept = (np.asarray(mask)[0, 0].T == 0).astype(bf16)
    in_maps = []
    for c in range(8):
        b, g = divmod(c, 4)
        q0 = g * FQ
        wq = Wqkv[:, q0 : q0 + FQ] * A2
        wk = Wqkv[:, D + q0 : D + q0 + FQ]
        wqk = np.concatenate([wq, wk], axis=1)
        bqk = np.concatenate(
            [bqkv[q0 : q0 + FQ] * A2, bqkv[D + q0 : D + q0 + FQ]]
        ).reshape(4, 128).T
        bv = np.broadcast_to(
            bqkv[2 * D + q0 : 2 * D + q0 + FQ].reshape(1, FQ), (128, FQ)
        )
        in_maps.append({
            "xt": np.ascontiguousarray(x[b].T).astype(bf16),
            "wqk": np.ascontiguousarray(wqk).astype(bf16),
            "wv": np.ascontiguousarray(
                Wqkv[:, 2 * D + q0 : 2 * D + q0 + FQ]).astype(bf16),
            "wp": np.ascontiguousarray(
                np.asarray(Wproj, np.float32)[q0 : q0 + FQ, :]),
            "bqk": np.ascontiguousarray(bqk).astype(np.float32),
            "bv": np.ascontiguousarray(bv).astype(bf16),
            "keept": keept,
        })
    return in_maps


_PROGRAM = None


def _get_program():
    global _PROGRAM
    if _PROGRAM is None:
        _PROGRAM = build_program(T_FULL)
    return _PROGRAM


def run_on_hw(in_maps, **kwargs):
    from concourse.bass_utils import run_bass_kernel_spmd

    nc = _get_program()
    return run_bass_kernel_spmd(nc, in_maps, list(range(8)), **kwargs)


def gather_output(results, bproj):
    parts = [results[c]["out"] for c in range(8)]
    out = np.stack([
        parts[0] + parts[1] + parts[2] + parts[3],
        parts[4] + parts[5] + parts[6] + parts[7],
    ])
    return (out + np.asarray(bproj, np.float32).reshape(1, 1, D)).astype(
        np.float32)


def kernel(x, mask, Wqkv, bqkv, Wproj, bproj):
    in_maps = shard_inputs(x, mask, Wqkv, bqkv, Wproj)
    res = run_on_hw(in_maps)
    return gather_output(res.results, bproj)
